# revision 2
# baseline (speedup 1.0000x reference)
"""Embedding gather kernel for Trainium2 (8 NeuronCores).

Problem: out[i] = select(cache_hit, weight_gpu[slot], weight_cpu[indices[i]]).
Since weight_gpu is constructed as weight_gpu = weight_cpu[gpu_cache_rows]
(a bitwise copy of table rows), the cache select is an identity:
out == weight_cpu[indices] exactly. So this is a pure 819200x64 f32 gather
from a 5M-row table.

Strategy (row-wise shard, host-side dispatch — the module's native
ShardingType.ROW_WISE with the all-to-alls done on the host):
 - Host dedups + sorts indices; owner core = idx // 625000. Each core gets
   its sorted unique local indices and its contiguous 625000-row table slab.
 - On-core: indices are pre-split (host) into 32768-row windows so they fit
   int16, then gathered window-by-window with gpsimd.dma_gather
   (InstDMAGatherAnt): each window's gather is split into 4 chunks spread
   over all 4 SWDGE queues (parallel Q7 descriptor generation — measured
   2.8x over a single queue), staged in SBUF, and streamed back to DRAM
   with large contiguous stores.
 - Host unswizzles the [128, slot] layout, expands duplicates, done.
"""

import numpy as np

P = 128
N_CORES = 8
WINDOW = 32768          # int16-addressable rows per gather window
PAD_QUANT = 128         # per-window capacity quantum (partition count)
N_QUEUES = 4            # SWDGE queues (ucode MAX_SWDGE_QUEUES)

_prog_cache: dict = {}


def _build_program(rows_per_core, d, capw):
    """Build + compile the per-core Bass program for window capacities capw."""
    import concourse.bacc as bacc
    import concourse.tile as tile
    from concourse import mybir

    s16_tot = sum(c // 16 for c in capw)
    s_tot = sum(c // 128 for c in capw)

    nc = bacc.Bacc(None, target_bir_lowering=False, num_swdge_queues=N_QUEUES)
    idx16 = nc.dram_tensor("idx16", [P, s16_tot], mybir.dt.int16, kind="ExternalInput")
    table = nc.dram_tensor(
        "table", [rows_per_core, d], mybir.dt.float32, kind="ExternalInput"
    )
    out = nc.dram_tensor("out", [P, s_tot, d], mybir.dt.float32, kind="ExternalOutput")

    with tile.TileContext(nc) as tc:
        with (
            tc.tile_pool(name="gpool", bufs=3) as gpool,
            tc.tile_pool(name="ipool", bufs=3) as ipool,
        ):
            off16 = 0
            offs = 0
            for w, cap in enumerate(capw):
                if cap == 0:
                    continue
                s16 = cap // 16
                s = cap // 128
                w_lo = w * WINDOW
                w_hi = min(w_lo + WINDOW, rows_per_core)
                it = ipool.tile([P, s16], mybir.dt.int16, tag="it")
                nc.sync.dma_start(out=it[:], in_=idx16[:, off16 : off16 + s16])
                gt = gpool.tile([P, s, d], mybir.dt.float32, tag="gt")
                # split the window's gather across all SWDGE queues
                chunk = -(-s // N_QUEUES) * 128
                for qi, o in enumerate(range(0, cap, chunk)):
                    cc = min(chunk, cap - o)
                    nc.gpsimd.dma_gather(
                        gt[:, o // 128 : (o + cc) // 128, :],
                        table[w_lo:w_hi, :],
                        it[:, o // 16 : (o + cc) // 16],
                        num_idxs=cc,
                        num_idxs_reg=cc,
                        elem_size=d,
                        single_packet=False,
                        queue_num=qi % N_QUEUES,
                    )
                nc.sync.dma_start(out=out[:, offs : offs + s, :], in_=gt[:, :, :])
                off16 += s16
                offs += s
    nc.compile()
    return nc


def _pack_idx16(local_sorted, win_bounds, capw):
    """Pack a core's sorted local indices into the wrapped int16 layout.

    Returns [P, sum(capw)//16] int16: window w's cap indices are wrapped as
    j -> [j%16, j//16] in partitions 0-15, replicated to all 8 groups of 16
    partitions (one copy per GPSIMD core). Padding repeats the last index.
    """
    cols = sum(c // 16 for c in capw)
    a = np.zeros((16, cols), dtype=np.int16)
    off16 = 0
    for w, cap in enumerate(capw):
        if cap == 0:
            continue
        s16 = cap // 16
        lo, hi = win_bounds[w], win_bounds[w + 1]
        seg = local_sorted[lo:hi] - w * WINDOW
        n = hi - lo
        buf = np.empty(cap, dtype=np.int16)
        buf[:n] = seg
        buf[n:] = seg[-1] if n > 0 else 0
        a[:, off16 : off16 + s16] = buf.reshape(s16, 16).T
        off16 += s16
    return np.tile(a, (8, 1))


def kernel(indices, weight_cpu, weight_gpu=None, gpu_cache_rows=None, **_):
    from concourse.bass_utils import run_bass_kernel_spmd

    idx = np.asarray(indices)
    table = np.ascontiguousarray(np.asarray(weight_cpu, dtype=np.float32))
    n = idx.shape[0]
    num_emb, d = table.shape
    rows_per_core = -(-num_emb // N_CORES)  # ceil

    # dedup (~8% fewer rows to gather); uidx is sorted ascending
    uidx, uinv = np.unique(idx.astype(np.int64, copy=False), return_inverse=True)
    nu = uidx.shape[0]

    # owner split (cores own contiguous row slabs)
    core_bounds = np.searchsorted(
        uidx, np.arange(N_CORES + 1, dtype=np.int64) * rows_per_core
    )

    n_win = -(-rows_per_core // WINDOW)
    win_edges = np.arange(n_win + 1, dtype=np.int64) * WINDOW
    all_bounds = []
    counts = np.zeros((N_CORES, n_win), dtype=np.int64)
    for c in range(N_CORES):
        lo, hi = core_bounds[c], core_bounds[c + 1]
        local = uidx[lo:hi] - c * rows_per_core
        wb = np.searchsorted(local, win_edges)
        all_bounds.append(wb)
        counts[c] = np.diff(wb)
    capw = tuple(
        int(-(-int(counts[:, w].max()) // PAD_QUANT) * PAD_QUANT) for w in range(n_win)
    )

    key = (rows_per_core, d, capw)
    nc = _prog_cache.get(key)
    if nc is None:
        nc = _prog_cache[key] = _build_program(rows_per_core, d, capw)

    # per-core inputs
    in_maps = []
    for c in range(N_CORES):
        lo, hi = core_bounds[c], core_bounds[c + 1]
        local = (uidx[lo:hi] - c * rows_per_core).astype(np.int32)
        idx16 = _pack_idx16(local, all_bounds[c], capw)
        slab_lo = c * rows_per_core
        slab_hi = min(slab_lo + rows_per_core, num_emb)
        slab = table[slab_lo:slab_hi]
        if slab.shape[0] < rows_per_core:  # pad last core's slab
            slab = np.concatenate(
                [slab, np.zeros((rows_per_core - slab.shape[0], d), np.float32)]
            )
        in_maps.append({"idx16": idx16, "table": slab})

    res = run_bass_kernel_spmd(nc, in_maps, core_ids=list(range(N_CORES)))

    # unswizzle (gathered row j of a window block sits at [j%128, j//128])
    s_off = np.concatenate([[0], np.cumsum([c // 128 for c in capw])])
    gathered = np.empty((nu, d), dtype=np.float32)
    pos = 0
    for c in range(N_CORES):
        o = res.results[c]["out"]  # [P, s_tot, d]
        for w, cap in enumerate(capw):
            cnt = int(counts[c, w])
            if cnt == 0:
                continue
            s = cap // 128
            block = o[:, s_off[w] : s_off[w] + s, :]  # [128, s, d]
            rows = block.transpose(1, 0, 2).reshape(-1, d)[:cnt]
            gathered[pos : pos + cnt] = rows
            pos += cnt
    assert pos == nu
    # expand duplicates back to the full lookup list
    return gathered[uinv]


# revision 3
# speedup vs baseline: 1.1123x; 1.1123x over previous
"""Embedding gather kernel for Trainium2 (8 NeuronCores).

Problem: out[i] = select(cache_hit, weight_gpu[slot], weight_cpu[indices[i]]).
Since weight_gpu is constructed as weight_gpu = weight_cpu[gpu_cache_rows]
(a bitwise copy of table rows), the cache select is an identity:
out == weight_cpu[indices] exactly. So this is a pure 819200x64 f32 gather
from a 5M-row table.

Strategy (row-wise shard, host-side dispatch — the module's native
ShardingType.ROW_WISE with the all-to-alls done on the host):
 - Host dedups + sorts indices; owner core = idx // 625000. Each core gets
   its sorted unique local indices and its contiguous 625000-row table slab.
 - On-core: indices are pre-split (host) into 32768-row windows so they fit
   int16, then gathered window-by-window with gpsimd.dma_gather
   (InstDMAGatherAnt): each window's gather is split into 4 chunks spread
   over all 4 SWDGE queues (parallel Q7 descriptor generation — measured
   2.8x over a single queue), staged in SBUF, and streamed back to DRAM
   with large contiguous stores.
 - Host unswizzles the [128, slot] layout, expands duplicates, done.
"""

import numpy as np

P = 128
N_CORES = 8
WINDOW = 32768          # int16-addressable rows per gather window
PAD_QUANT = 128         # per-window capacity quantum (partition count)
N_QUEUES = 4            # SWDGE queues (ucode MAX_SWDGE_QUEUES)

_prog_cache: dict = {}


def _build_program(rows_per_core, d, capw):
    """Build + compile the per-core Bass program for window capacities capw."""
    import concourse.bacc as bacc
    import concourse.tile as tile
    from concourse import mybir

    s16_tot = sum(c // 16 for c in capw)
    s_tot = sum(c // 128 for c in capw)

    nc = bacc.Bacc(None, target_bir_lowering=False, num_swdge_queues=N_QUEUES)
    idx16 = nc.dram_tensor("idx16", [P, s16_tot], mybir.dt.int16, kind="ExternalInput")
    table = nc.dram_tensor(
        "table", [rows_per_core, d], mybir.dt.float32, kind="ExternalInput"
    )
    out = nc.dram_tensor("out", [P, s_tot, d], mybir.dt.float32, kind="ExternalOutput")

    with tile.TileContext(nc) as tc:
        with (
            tc.tile_pool(name="gpool", bufs=3) as gpool,
            tc.tile_pool(name="ipool", bufs=3) as ipool,
        ):
            off16 = 0
            offs = 0
            for w, cap in enumerate(capw):
                if cap == 0:
                    continue
                s16 = cap // 16
                s = cap // 128
                w_lo = w * WINDOW
                w_hi = min(w_lo + WINDOW, rows_per_core)
                it = ipool.tile([P, s16], mybir.dt.int16, tag="it")
                nc.sync.dma_start(out=it[:], in_=idx16[:, off16 : off16 + s16])
                gt = gpool.tile([P, s, d], mybir.dt.float32, tag="gt")
                # split the window's gather into 8 chunks cycling the 4 SWDGE
                # queues (finer interleave balances queue tails)
                chunk = -(-s // 8) * 128
                for qi, o in enumerate(range(0, cap, chunk)):
                    cc = min(chunk, cap - o)
                    nc.gpsimd.dma_gather(
                        gt[:, o // 128 : (o + cc) // 128, :],
                        table[w_lo:w_hi, :],
                        it[:, o // 16 : (o + cc) // 16],
                        num_idxs=cc,
                        num_idxs_reg=cc,
                        elem_size=d,
                        single_packet=False,
                        queue_num=qi % N_QUEUES,
                    )
                nc.sync.dma_start(out=out[:, offs : offs + s, :], in_=gt[:, :, :])
                off16 += s16
                offs += s
    nc.compile()
    return nc


def _pack_idx16(local_sorted, win_bounds, capw):
    """Pack a core's sorted local indices into the wrapped int16 layout.

    Returns [P, sum(capw)//16] int16: window w's cap indices are wrapped as
    j -> [j%16, j//16] in partitions 0-15, replicated to all 8 groups of 16
    partitions (one copy per GPSIMD core). Padding repeats the last index.
    """
    cols = sum(c // 16 for c in capw)
    a = np.zeros((16, cols), dtype=np.int16)
    off16 = 0
    for w, cap in enumerate(capw):
        if cap == 0:
            continue
        s16 = cap // 16
        lo, hi = win_bounds[w], win_bounds[w + 1]
        seg = local_sorted[lo:hi] - w * WINDOW
        n = hi - lo
        buf = np.empty(cap, dtype=np.int16)
        buf[:n] = seg
        buf[n:] = seg[-1] if n > 0 else 0
        a[:, off16 : off16 + s16] = buf.reshape(s16, 16).T
        off16 += s16
    return np.tile(a, (8, 1))


def kernel(indices, weight_cpu, weight_gpu=None, gpu_cache_rows=None, **_):
    from concourse.bass_utils import run_bass_kernel_spmd

    idx = np.asarray(indices)
    table = np.ascontiguousarray(np.asarray(weight_cpu, dtype=np.float32))
    n = idx.shape[0]
    num_emb, d = table.shape
    rows_per_core = -(-num_emb // N_CORES)  # ceil

    # dedup (~8% fewer rows to gather); uidx is sorted ascending
    uidx, uinv = np.unique(idx.astype(np.int64, copy=False), return_inverse=True)
    nu = uidx.shape[0]

    # owner split (cores own contiguous row slabs)
    core_bounds = np.searchsorted(
        uidx, np.arange(N_CORES + 1, dtype=np.int64) * rows_per_core
    )

    n_win = -(-rows_per_core // WINDOW)
    win_edges = np.arange(n_win + 1, dtype=np.int64) * WINDOW
    all_bounds = []
    counts = np.zeros((N_CORES, n_win), dtype=np.int64)
    for c in range(N_CORES):
        lo, hi = core_bounds[c], core_bounds[c + 1]
        local = uidx[lo:hi] - c * rows_per_core
        wb = np.searchsorted(local, win_edges)
        all_bounds.append(wb)
        counts[c] = np.diff(wb)
    capw = tuple(
        int(-(-int(counts[:, w].max()) // PAD_QUANT) * PAD_QUANT) for w in range(n_win)
    )

    key = (rows_per_core, d, capw)
    nc = _prog_cache.get(key)
    if nc is None:
        nc = _prog_cache[key] = _build_program(rows_per_core, d, capw)

    # per-core inputs
    in_maps = []
    for c in range(N_CORES):
        lo, hi = core_bounds[c], core_bounds[c + 1]
        local = (uidx[lo:hi] - c * rows_per_core).astype(np.int32)
        idx16 = _pack_idx16(local, all_bounds[c], capw)
        slab_lo = c * rows_per_core
        slab_hi = min(slab_lo + rows_per_core, num_emb)
        slab = table[slab_lo:slab_hi]
        if slab.shape[0] < rows_per_core:  # pad last core's slab
            slab = np.concatenate(
                [slab, np.zeros((rows_per_core - slab.shape[0], d), np.float32)]
            )
        in_maps.append({"idx16": idx16, "table": slab})

    res = run_bass_kernel_spmd(nc, in_maps, core_ids=list(range(N_CORES)))

    # unswizzle (gathered row j of a window block sits at [j%128, j//128])
    s_off = np.concatenate([[0], np.cumsum([c // 128 for c in capw])])
    gathered = np.empty((nu, d), dtype=np.float32)
    pos = 0
    for c in range(N_CORES):
        o = res.results[c]["out"]  # [P, s_tot, d]
        for w, cap in enumerate(capw):
            cnt = int(counts[c, w])
            if cnt == 0:
                continue
            s = cap // 128
            block = o[:, s_off[w] : s_off[w] + s, :]  # [128, s, d]
            rows = block.transpose(1, 0, 2).reshape(-1, d)[:cnt]
            gathered[pos : pos + cnt] = rows
            pos += cnt
    assert pos == nu
    # expand duplicates back to the full lookup list
    return gathered[uinv]


# revision 4
# speedup vs baseline: 1.2145x; 1.0918x over previous
"""Embedding gather kernel for Trainium2 (8 NeuronCores).

Problem: out[i] = select(cache_hit, weight_gpu[slot], weight_cpu[indices[i]]).
Since weight_gpu is constructed as weight_gpu = weight_cpu[gpu_cache_rows]
(a bitwise copy of table rows), the cache select is an identity:
out == weight_cpu[indices] exactly. So this is a pure 819200x64 f32 gather
from a 5M-row table.

Strategy (row-wise shard, host-side dispatch — the module's native
ShardingType.ROW_WISE with the all-to-alls done on the host):
 - Host dedups + sorts indices; owner core = idx // 625000. Each core gets
   its sorted unique local indices and its contiguous 625000-row table slab.
 - On-core: indices are pre-split (host) into 32768-row windows so they fit
   int16, then gathered window-by-window with gpsimd.dma_gather
   (InstDMAGatherAnt): each window's gather is split into 4 chunks spread
   over all 4 SWDGE queues (parallel Q7 descriptor generation — measured
   2.8x over a single queue), staged in SBUF, and streamed back to DRAM
   with large contiguous stores.
 - Host unswizzles the [128, slot] layout, expands duplicates, done.
"""

import numpy as np

P = 128
N_CORES = 8
WINDOW = 32768          # int16-addressable rows per gather window
PAD_QUANT = 128         # per-window capacity quantum (partition count)
N_QUEUES = 4            # SWDGE queues (ucode MAX_SWDGE_QUEUES)

_prog_cache: dict = {}


def _build_program(rows_per_core, d, capw):
    """Build + compile the per-core Bass program for window capacities capw."""
    import concourse.bacc as bacc
    import concourse.tile as tile
    from concourse import mybir

    s16_tot = sum(c // 16 for c in capw)
    s_tot = sum(c // 128 for c in capw)

    nc = bacc.Bacc(None, target_bir_lowering=False, num_swdge_queues=N_QUEUES)
    idx16 = nc.dram_tensor("idx16", [P, s16_tot], mybir.dt.int16, kind="ExternalInput")
    table = nc.dram_tensor(
        "table", [rows_per_core, d], mybir.dt.float32, kind="ExternalInput"
    )
    out = nc.dram_tensor("out", [P, s_tot, d], mybir.dt.float32, kind="ExternalOutput")

    with tile.TileContext(nc) as tc:
        with (
            tc.tile_pool(name="gpool", bufs=3) as gpool,
            tc.tile_pool(name="ipool", bufs=3) as ipool,
        ):
            off16 = 0
            offs = 0
            for w, cap in enumerate(capw):
                if cap == 0:
                    continue
                s16 = cap // 16
                s = cap // 128
                w_lo = w * WINDOW
                w_hi = min(w_lo + WINDOW, rows_per_core)
                it = ipool.tile([P, s16], mybir.dt.int16, tag="it")
                nc.sync.dma_start(out=it[:], in_=idx16[:, off16 : off16 + s16])
                gt = gpool.tile([P, s, d], mybir.dt.float32, tag="gt")
                # split the window's gather into 8 chunks cycling the 4 SWDGE
                # queues (finer interleave balances queue tails)
                chunk = -(-s // 8) * 128
                for qi, o in enumerate(range(0, cap, chunk)):
                    cc = min(chunk, cap - o)
                    nc.gpsimd.dma_gather(
                        gt[:, o // 128 : (o + cc) // 128, :],
                        table[w_lo:w_hi, :],
                        it[:, o // 16 : (o + cc) // 16],
                        num_idxs=cc,
                        num_idxs_reg=cc,
                        elem_size=d,
                        # single-packet descriptor gen is much lower-variance;
                        # only legal while the chunk fits the 16KB ring
                        single_packet=(cc <= 1024),
                        queue_num=qi % N_QUEUES,
                    )
                nc.sync.dma_start(out=out[:, offs : offs + s, :], in_=gt[:, :, :])
                off16 += s16
                offs += s
    nc.compile()
    return nc


def _pack_idx16(local_sorted, win_bounds, capw):
    """Pack a core's sorted local indices into the wrapped int16 layout.

    Returns [P, sum(capw)//16] int16: window w's cap indices are wrapped as
    j -> [j%16, j//16] in partitions 0-15, replicated to all 8 groups of 16
    partitions (one copy per GPSIMD core). Padding repeats the last index.
    """
    cols = sum(c // 16 for c in capw)
    a = np.zeros((16, cols), dtype=np.int16)
    off16 = 0
    for w, cap in enumerate(capw):
        if cap == 0:
            continue
        s16 = cap // 16
        lo, hi = win_bounds[w], win_bounds[w + 1]
        seg = local_sorted[lo:hi] - w * WINDOW
        n = hi - lo
        buf = np.empty(cap, dtype=np.int16)
        buf[:n] = seg
        buf[n:] = seg[-1] if n > 0 else 0
        a[:, off16 : off16 + s16] = buf.reshape(s16, 16).T
        off16 += s16
    return np.tile(a, (8, 1))


def kernel(indices, weight_cpu, weight_gpu=None, gpu_cache_rows=None, **_):
    from concourse.bass_utils import run_bass_kernel_spmd

    idx = np.asarray(indices)
    table = np.ascontiguousarray(np.asarray(weight_cpu, dtype=np.float32))
    n = idx.shape[0]
    num_emb, d = table.shape
    rows_per_core = -(-num_emb // N_CORES)  # ceil

    # dedup (~8% fewer rows to gather); uidx is sorted ascending
    uidx, uinv = np.unique(idx.astype(np.int64, copy=False), return_inverse=True)
    nu = uidx.shape[0]

    # owner split (cores own contiguous row slabs)
    core_bounds = np.searchsorted(
        uidx, np.arange(N_CORES + 1, dtype=np.int64) * rows_per_core
    )

    n_win = -(-rows_per_core // WINDOW)
    win_edges = np.arange(n_win + 1, dtype=np.int64) * WINDOW
    all_bounds = []
    counts = np.zeros((N_CORES, n_win), dtype=np.int64)
    for c in range(N_CORES):
        lo, hi = core_bounds[c], core_bounds[c + 1]
        local = uidx[lo:hi] - c * rows_per_core
        wb = np.searchsorted(local, win_edges)
        all_bounds.append(wb)
        counts[c] = np.diff(wb)
    capw = tuple(
        int(-(-int(counts[:, w].max()) // PAD_QUANT) * PAD_QUANT) for w in range(n_win)
    )

    key = (rows_per_core, d, capw)
    nc = _prog_cache.get(key)
    if nc is None:
        nc = _prog_cache[key] = _build_program(rows_per_core, d, capw)

    # per-core inputs
    in_maps = []
    for c in range(N_CORES):
        lo, hi = core_bounds[c], core_bounds[c + 1]
        local = (uidx[lo:hi] - c * rows_per_core).astype(np.int32)
        idx16 = _pack_idx16(local, all_bounds[c], capw)
        slab_lo = c * rows_per_core
        slab_hi = min(slab_lo + rows_per_core, num_emb)
        slab = table[slab_lo:slab_hi]
        if slab.shape[0] < rows_per_core:  # pad last core's slab
            slab = np.concatenate(
                [slab, np.zeros((rows_per_core - slab.shape[0], d), np.float32)]
            )
        in_maps.append({"idx16": idx16, "table": slab})

    res = run_bass_kernel_spmd(nc, in_maps, core_ids=list(range(N_CORES)))

    # unswizzle (gathered row j of a window block sits at [j%128, j//128])
    s_off = np.concatenate([[0], np.cumsum([c // 128 for c in capw])])
    gathered = np.empty((nu, d), dtype=np.float32)
    pos = 0
    for c in range(N_CORES):
        o = res.results[c]["out"]  # [P, s_tot, d]
        for w, cap in enumerate(capw):
            cnt = int(counts[c, w])
            if cnt == 0:
                continue
            s = cap // 128
            block = o[:, s_off[w] : s_off[w] + s, :]  # [128, s, d]
            rows = block.transpose(1, 0, 2).reshape(-1, d)[:cnt]
            gathered[pos : pos + cnt] = rows
            pos += cnt
    assert pos == nu
    # expand duplicates back to the full lookup list
    return gathered[uinv]
